# revision 37
# baseline (speedup 1.0000x reference)
"""GAT attention layer (B=8, N=2048, C=512) on 8 TRN2 NeuronCores.

Data-parallel over B: core b handles graph b.
Per-core math (x: [N,C], w: [C,C], a: [2C,1]):
    wa_t = w @ a_t                      (t=0,1)       [C]
    s_t  = x @ wa_t                                   [N]
    z_ji = s1_i + s2_j                 (transposed scores, j=source)
    p_ji = exp(leaky_relu(z)) = max(exp(z), exp(.2 s1)exp(.2 s2))
    r_i  = sum_j p_ji
    out  = (p^T @ x) / r
No softmax max-subtraction needed: z in [-25, 25] so exp stays in fp32 range.
Scores kept transposed [j, i] so p slices serve directly as matmul stationary
operands (out[i,c] = sum_j p[j,i] x[j,c]) and r comes from rhs=ones matmuls.

exp(leaky(z)) is factored: the exp(z) branch is ONE ACT pass per block
(exp table, bias=s2col); the exp(0.2 z) branch factors into an outer
product u'_i v'_j of two precomputed exp vectors, fused with the max into
ONE DVE scalar_tensor_tensor pass (bf16 2x mode).  This halves ACT work
vs computing both exp branches on ACT.

x is loaded as bf16 directly (SWDGE cast-DMA, 4x 1MB batches): the PV
matmuls need bf16 anyway and the s1/s2 row-dots run fine on bf16, so the
fp32 copy of x never has to exist on-chip.  Partition->free transposes
(s1, wa, exp(.2 s1)) are done with DVE 32x32 block transposes so the
DRAM scatter side of each broadcast is contiguous runs instead of
4-byte-per-descriptor writes (which cost ~10us in write receipts).
"""

import sys

import numpy as np

if "/opt/trn_rl_repo" not in sys.path:
    sys.path.insert(0, "/opt/trn_rl_repo")

B, N, C = 8, 2048, 512
P = 128
NJ = N // P  # 16 source-node blocks
NG = 4  # x loaded in 4 groups of 4 blocks
ALPHA = 0.2  # leaky_relu slope
NCH = C // P  # 4 channel chunks
# PSUM: 8 banks of [128, 512] fp32. Chunk groups sized so each group's output
# accumulators (one bank per i-chunk) plus the shared r bank fit in 8.
GROUPS = [(0, 7), (7, 14), (14, 16)]

_CACHE = {}


def _build():
    from contextlib import ExitStack

    import concourse.bacc as bacc
    import concourse.bass as bass
    import concourse.tile as tile
    from concourse import mybir

    fp32 = mybir.dt.float32
    bf16 = mybir.dt.bfloat16
    AF = mybir.ActivationFunctionType
    OP = mybir.AluOpType

    nc = bacc.Bacc("TRN2", target_bir_lowering=False)
    x_d = nc.dram_tensor("x", [N, C], fp32, kind="ExternalInput")
    w_d = nc.dram_tensor("w", [C, C], fp32, kind="ExternalInput")
    a_d = nc.dram_tensor("a", [2 * C, 1], fp32, kind="ExternalInput")
    o_d = nc.dram_tensor("o", [N, C], fp32, kind="ExternalOutput")

    with ExitStack() as ctx:
        tc = ctx.enter_context(tile.TileContext(nc))
        const = ctx.enter_context(tc.tile_pool(name="const", bufs=1))
        wpool = ctx.enter_context(tc.tile_pool(name="w", bufs=NCH))
        xbfp = ctx.enter_context(tc.tile_pool(name="xbf", bufs=NG))
        ppool = ctx.enter_context(tc.tile_pool(name="p", bufs=NJ))
        eab = ctx.enter_context(tc.tile_pool(name="eab", bufs=8))
        scr = ctx.enter_context(tc.tile_pool(name="scr", bufs=6))
        osb = ctx.enter_context(tc.tile_pool(name="osb", bufs=3))
        dram = ctx.enter_context(tc.tile_pool(name="dram", bufs=1, space="DRAM"))
        ps_out = ctx.enter_context(tc.tile_pool(name="ps_out", bufs=7, space="PSUM"))
        ps_r = ctx.enter_context(tc.tile_pool(name="ps_r", bufs=1, space="PSUM"))

        # --- persistent small tiles -------------------------------------
        s1col = const.tile([P, 32], fp32)  # s1[128j+p] at [p, j] (j<16)
        s1colb = const.tile([P, 32], bf16)
        s1T = const.tile([P, 32], bf16)  # 32x32-block transpose of s1colb
        upT = const.tile([P, 32], bf16)  # exp(ALPHA * s1T)
        s2col = const.tile([P, NJ], fp32)
        vpcol = const.tile([P, NJ], fp32)  # exp(ALPHA*s2) at [p, j]
        wa12 = const.tile([P, 32], fp32)  # wa_t[128q+p] at [p, t*NCH+q]
        wa12b = const.tile([P, 32], bf16)
        waT = const.tile([P, 32], bf16)  # 32x32-block transpose of wa12b
        abc = const.tile([P, 2, C], bf16)  # a rows broadcast to 128 parts
        wab = const.tile([P, 2, C], bf16)  # wa rows broadcast to 128 parts
        s1b = const.tile([P, N], bf16)  # s1 broadcast to 128 parts
        ubp = const.tile([P, N], bf16)  # exp(ALPHA*s1) broadcast
        ones_bf = const.tile([P, 1], bf16)
        ones128 = const.tile([P, P], bf16)
        rinv = const.tile([P, NJ], fp32)
        dummy = const.tile([P, 1], fp32)

        scratch_wa = dram.tile([2 * C], bf16)
        scratch_s1 = dram.tile([N], bf16)
        scratch_up = dram.tile([N], bf16)

        nc.vector.memset(ones_bf[:], 1.0)
        warm_rhs = const.tile([P, C], bf16)
        nc.vector.memset(warm_rhs[:], 0.0)
        nc.vector.memset(ones128[:], 1.0)
        nc.vector.memset(dummy[:], 0.0)

        # Preload the exp ACT table as early as possible (one-time ~2.7us).
        nc.scalar.activation(dummy[:], dummy[:], AF.Exp, bias=0.0, scale=1.0)

        warm_ps = ps_r.tile([P, C], fp32, tag="rps", name="warm_ps")

        def warm_mm():
            nc.tensor.matmul(
                warm_ps[0:1, :],
                lhsT=ones_bf[:],
                rhs=warm_rhs[:],
                start=True,
                stop=True,
                skip_group_check=True,
            )

        for _ in range(16):
            warm_mm()
        # Clock-gated warm bursts: keep the PE HAM activity monitor fed
        # through the ~45us head phase so group 0 starts at K=8/8 instead of
        # paying a ~4us half-clock ramp.  The last gate (42us) is safely
        # before the first PV matmul (~48us), so these cannot delay it.
        for tus in range(15, 45, 3):
            with tc.tile_wait_until(tus * 1e-3):
                for _ in range(4):
                    warm_mm()

        # --- w loads first (gates the wa -> wab chain) -------------------
        wt = []
        for q in range(NCH):
            t = wpool.tile([P, C], fp32, tag="w")
            eng = nc.sync if q % 2 == 0 else nc.scalar
            eng.dma_start(t[:], w_d[q * P : (q + 1) * P, :])
            wt.append(t)

        # --- a -> abc (partition-broadcast DMA straight from DRAM) ------
        a_rows = a_d[:, 0].rearrange("(t c) -> t c", t=2)  # [2, C]
        nc.gpsimd.dma_start(
            out=abc[:],
            in_=bass.AP(
                tensor=a_rows.tensor,
                offset=a_rows.offset,
                ap=[[0, P]] + list(a_rows.ap),
            ),
        )

        # --- x loads: 4x 1MB SWDGE cast-DMAs straight to bf16 ------------
        # xq[g][p, jj, c] = x[512g + 128jj + p, c];  block j = xq[j//4][:, j%4, :]
        xq = []
        for g in range(NG):
            t = xbfp.tile([P, 4, C], bf16, tag="xbf")
            src = x_d[512 * g : 512 * (g + 1), :].rearrange(
                "(jj p) c -> p jj c", p=P
            )
            nc.gpsimd.dma_start(out=t[:], in_=src)
            xq.append(t)

        def xbf(j):
            return xq[j // 4][:, j % 4, :]

        xf = xbf

        def scatter_col32(src_t, dst_scratch, nrow, f0=0):
            # src_t is the 32x32-block transpose of a [P, 32] column tile:
            # src_t[32b + f, pp] = col[32b + pp, f].  Value for (f, block j=f)
            # of original col index goes to dram[128f + 32b + pp] -- one
            # contiguous 32-element run per (b, f): 4 HWDGE DMAs of nrow
            # descriptors each, split across the two HWDGE queues so the
            # kicks and write receipts overlap.
            for b in range(4):
                eng = nc.sync if b % 2 == 0 else nc.scalar
                eng.dma_start(
                    bass.AP(
                        tensor=dst_scratch.tensor,
                        offset=dst_scratch.offset + 32 * b + P * f0,
                        ap=[[P, nrow], [1, 32]],
                    ),
                    src_t[32 * b + f0 : 32 * b + f0 + nrow, :],
                )

        # --- wa = w @ a via DVE row-dots, then DRAM round-trip broadcast -
        for q in range(NCH):
            for t in range(2):
                s = scr.tile([P, C], fp32, tag="ttr")
                nc.vector.scalar_tensor_tensor(
                    out=s[:],
                    in0=wt[q][:],
                    scalar=0.0,
                    in1=abc[:, t, :],
                    op0=OP.add,
                    op1=OP.mult,
                    accum_out=wa12[:, t * NCH + q : t * NCH + q + 1],
                )
        # wa12 -> bf16 -> transpose on DVE -> contiguous DRAM scatter at
        # [128*(4t+q) + p] = [t*C + c], then broadcast back as rows.
        nc.vector.tensor_copy(wa12b[:], wa12[:])
        nc.vector.transpose(waT[:], wa12b[:])
        scatter_col32(waT, scratch_wa[:], 2 * NCH)
        for t in range(2):
            eng = nc.sync if t == 0 else nc.scalar
            eng.dma_start(
                wab[:, t, :],
                bass.AP(
                    tensor=scratch_wa[:].tensor,
                    offset=scratch_wa[:].offset + t * C,
                    ap=[[0, P], [1, C]],
                ),
            )

        # --- s1 row-dots (chase the x DMAs).  accum_out forces DVE ops to
        # 1x mode, so odd blocks instead run the product as a plain
        # tensor_tensor (bf16 2x) and let the otherwise-idle ACT engine do
        # the summation with its accumulator.  The dots and the
        # scatter/broadcast chain are processed in two i-halves so the
        # first half of s1b (all that blocks' scores for i<1024 need) ships
        # ~6us earlier and the PV group-0 matmuls start sooner.
        def emit_s1_dot(j):
            if j % 2 == 0:
                s = scr.tile([P, C], bf16, tag="ttr")
                nc.vector.scalar_tensor_tensor(
                    out=s[:],
                    in0=xf(j),
                    scalar=0.0,
                    in1=wab[:, 0, :],
                    op0=OP.add,
                    op1=OP.mult,
                    accum_out=s1col[:, j : j + 1],
                )
            else:
                prod = scr.tile([P, C], bf16, tag="ttr", name=f"s1prod_{j}")
                nc.vector.tensor_tensor(
                    out=prod[:], in0=xf(j), in1=wab[:, 0, :], op=OP.mult
                )
                aout = scr.tile([P, C], bf16, tag="ttr", name=f"s1aout_{j}")
                nc.scalar.activation(
                    aout[:], prod[:], AF.Copy, bias=0.0, scale=1.0,
                    accum_out=s1col[:, j : j + 1],
                )

        for h in range(2):
            for j in range(8 * h, 8 * h + 8):
                emit_s1_dot(j)
            nc.vector.tensor_copy(s1colb[:], s1col[:])
            nc.vector.transpose(s1T[:], s1colb[:])
            nc.scalar.activation(upT[:], s1T[:], AF.Exp, bias=0.0, scale=ALPHA)
            scatter_col32(s1T, scratch_s1[:], 8, f0=8 * h)
            scatter_col32(upT, scratch_up[:], 8, f0=8 * h)
            eng = nc.sync if h == 0 else nc.scalar
            eng.dma_start(
                s1b[:, h * 1024 : (h + 1) * 1024],
                bass.AP(
                    tensor=scratch_s1[:].tensor,
                    offset=scratch_s1[:].offset + h * 1024,
                    ap=[[0, P], [1, 1024]],
                ),
            )
            eng2 = nc.scalar if h == 0 else nc.sync
            eng2.dma_start(
                ubp[:, h * 1024 : (h + 1) * 1024],
                bass.AP(
                    tensor=scratch_up[:].tensor,
                    offset=scratch_up[:].offset + h * 1024,
                    ap=[[0, P], [1, 1024]],
                ),
            )

        # s2 row-dots, split DVE / GPSIMD (GPSIMD is idle during scoring).
        def emit_s2(j):
            s = scr.tile([P, C], bf16, tag="ttr", name=f"s2scr_{j}")
            eng = nc.vector
            eng.scalar_tensor_tensor(
                out=s[:],
                in0=xf(j),
                scalar=0.0,
                in1=wab[:, 1, :],
                op0=OP.add,
                op1=OP.mult,
                accum_out=s2col[:, j : j + 1],
            )

        for j in range(4):
            emit_s2(j)

        H = 1024  # i-half boundary

        # --- scores: p_j[j_local, i] = max(exp(s1_i+s2_j), u'_i v'_j) ----
        # Blocks 0-3 produce their i<1024 half first (needs only the first
        # half of s1b/ubp), which is all the group-0 matmuls (chunks 0-6)
        # consume; their second halves are deferred to the end and groups
        # 1-2 visit these blocks last.
        pt = []
        eas = []
        for j in range(4):
            ea = eab.tile([P, N], bf16, tag="ea")
            nc.scalar.activation(
                ea[:, :H], s1b[:, :H], AF.Exp,
                bias=s2col[:, j : j + 1], scale=1.0,
            )
            if j == 0:
                nc.scalar.activation(
                    vpcol[:, 0:4], s2col[:, 0:4], AF.Exp, bias=0.0, scale=ALPHA
                )
            p = ppool.tile([P, N], bf16, tag="p")
            nc.vector.scalar_tensor_tensor(
                out=p[:, :H],
                in0=ubp[:, :H],
                scalar=vpcol[:, j : j + 1],
                in1=ea[:, :H],
                op0=OP.mult,
                op1=OP.max,
            )
            pt.append(p)
            eas.append(ea)
            emit_s2(j + 4)

        nc.scalar.activation(
            vpcol[:, 4:8], s2col[:, 4:8], AF.Exp, bias=0.0, scale=ALPHA
        )
        for j in range(4, NJ):
            ea = eab.tile([P, N], bf16, tag="ea")
            nc.scalar.activation(
                ea[:], s1b[:], AF.Exp, bias=s2col[:, j : j + 1], scale=1.0
            )
            p = ppool.tile([P, N], bf16, tag="p")
            nc.vector.scalar_tensor_tensor(
                out=p[:],
                in0=ubp[:],
                scalar=vpcol[:, j : j + 1],
                in1=ea[:],
                op0=OP.mult,
                op1=OP.max,
            )
            pt.append(p)
            if j + 4 < NJ:
                emit_s2(j + 4)
            if j % 4 == 3 and j + 5 <= NJ:
                # vp for blocks j+1..j+4; their s2 dots are all emitted.
                nc.scalar.activation(
                    vpcol[:, j + 1 : j + 5], s2col[:, j + 1 : j + 5], AF.Exp,
                    bias=0.0, scale=ALPHA,
                )

        # deferred second halves of blocks 0-3
        for j in range(4):
            nc.scalar.activation(
                eas[j][:, H:], s1b[:, H:], AF.Exp,
                bias=s2col[:, j : j + 1], scale=1.0,
            )
            nc.vector.scalar_tensor_tensor(
                out=pt[j][:, H:],
                in0=ubp[:, H:],
                scalar=vpcol[:, j : j + 1],
                in1=eas[j][:, H:],
                op0=OP.mult,
                op1=OP.max,
            )

        # --- PV + r + normalize, in PSUM-sized chunk groups --------------
        for g0, g1 in GROUPS:
            nk = g1 - g0
            outps = [
                ps_out.tile([P, C], fp32, tag="ops", name=f"ops_{g0}_{ki}")
                for ki in range(nk)
            ]
            rps = ps_r.tile([P, C], fp32, tag="rps")
            if g0 == 0:
                nc.tensor.matmul(
                    rps[:, :],
                    lhsT=ones128[:],
                    rhs=warm_rhs[:],
                    start=True,
                    stop=False,
                    skip_group_check=True,
                )
            jorder = list(range(NJ)) if g0 == 0 else (
                list(range(4, NJ)) + [0, 1, 2, 3]
            )
            for jpos, j in enumerate(jorder):
                first, last = jpos == 0, jpos == NJ - 1
                for ki, k in enumerate(range(g0, g1)):
                    lhs = pt[j][:, k * P : (k + 1) * P]
                    # start=True clears the WHOLE bank's has_written bits, so
                    # only the very first matmul into this bank may set it;
                    # later first-touches per element overwrite (bit clear)
                    # and the rest accumulate.  The tiny r matmul goes FIRST
                    # so both LDWEIGHTS of the pair prefetch during the
                    # previous chunk's 512-wide out matmul.
                    nc.tensor.matmul(
                        rps[:, ki : ki + 1],
                        lhsT=lhs,
                        rhs=ones_bf[:],
                        start=(first and ki == 0) and g0 != 0,
                        stop=last,
                        skip_group_check=True,
                    )
                    nc.tensor.matmul(
                        outps[ki][:], lhsT=lhs, rhs=xbf(j), start=first, stop=last
                    )
            nc.vector.reciprocal(rinv[:, g0:g1], rps[:, :nk])
            for ki, k in enumerate(range(g0, g1)):
                # Alternate the normalize between DVE and ACT (idle once the
                # score stream ends) so PSUM banks release ~2x faster and the
                # next group's matmuls stall less.
                ob = osb.tile([P, C], fp32, tag="ob")
                if ki % 2 == 0:
                    nc.vector.tensor_scalar_mul(
                        ob[:], outps[ki][:], rinv[:, k : k + 1]
                    )
                else:
                    nc.scalar.activation(
                        ob[:], outps[ki][:], AF.Copy,
                        bias=0.0, scale=rinv[:, k : k + 1],
                    )
                nc.sync.dma_start(o_d[k * P : (k + 1) * P, :], ob[:])

    nc.compile()
    return nc


def _get_nc():
    if "nc" not in _CACHE:
        _CACHE["nc"] = _build()
    return _CACHE["nc"]


def _run(inputs, trace=False, tmpdir=None):
    from concourse.bass_utils import run_bass_kernel_spmd

    nc = _get_nc()
    x = np.ascontiguousarray(np.asarray(inputs["x"], dtype=np.float32))
    w = np.ascontiguousarray(np.asarray(inputs["w"], dtype=np.float32))
    a = np.ascontiguousarray(np.asarray(inputs["a"], dtype=np.float32))
    core_ids = list(range(B))
    in_maps = [{"x": x[b], "w": w, "a": a} for b in core_ids]
    res = run_bass_kernel_spmd(nc, in_maps, core_ids, trace=trace, tmpdir=tmpdir)
    out = np.stack([res.results[b]["o"] for b in core_ids], axis=0)
    return out, res


def kernel(**inputs) -> np.ndarray:
    out, _ = _run(inputs, trace=False)
    return out
